# revision 9
# baseline (speedup 1.0000x reference)
"""Trainium2 Bass kernel for nn_BiEncoder_63024350101542 (segment_reduce).

Computes, per batch row b of vector_all [B=64, L=512, D=1024]:
    mask[b,j] = (j > first_idx(ids[b]==1)) & (j < first_idx(ids[b]==2))
    span_max  = max over masked rows (fallback: CLS row 0 when mask empty)
    out[b]    = cls + mu * span_max

Only rows inside the mention span (plus the CLS row) can affect the
output, so the host shards each core's inputs as packed span windows
instead of full batches: batches are ranked by span length and dealt
round-robin into per-core slots (rank-banded), so slot j holds the same
row count on every core and one SPMD program serves all 8 cores.  Slots
are padded to a multiple of 32 rows by cycling rows of the same span
(duplicates don't change a max); empty spans are filled with the CLS
row, which makes the empty-span fallback (vec = cls) exact with no
masking at all.

Per core the padded span rows form one contiguous buffer R, streamed in
128-row slices.  Each slice gets a transpose-fused 32x32 max-reduce
(DVE) collapsing its partition groups, a PE transpose lands the per-
group maxima in PSUM laid out so every slot owns a contiguous group
range, and one tensor_reduce per slot finishes the max.  Finally
out = cls + mu * vec.
"""

import os
import sys

import numpy as np

for _p in ("/root/.axon_site/_ro/trn_rl_repo", "/opt/trn_rl_repo"):
    if _p not in sys.path and os.path.isdir(_p):
        sys.path.append(_p)

import concourse.bacc as bacc
import concourse.bass as bass
import concourse.mybir as mybir
import concourse.tile as tile
from concourse.bass_utils import run_bass_kernel_spmd

F32 = mybir.dt.float32
X = mybir.AxisListType.X
Alu = mybir.AluOpType

B, L, D = 64, 512, 1024
NCORES = 8
NB = B // NCORES           # batches (slots) per core
MENTION_START, MENTION_END = 1, 2

KCOLS = 392                # konst: ident 0:128, mu 128, cls rows at 136:392
CLS_OFF = 136


# ---------------------------------------------------------------- plan

def compute_spans(ids):
    """Per batch: span start s and length n (rows s..s+n-1 are masked in)."""
    ids = np.asarray(ids)
    is1 = ids == MENTION_START
    is2 = ids == MENTION_END
    first1 = np.where(is1.any(1), is1.argmax(1), L).astype(np.int64)
    first2 = np.where(is2.any(1), is2.argmax(1), L).astype(np.int64)
    s = first1 + 1
    n = np.maximum(0, first2 - s)
    return s, n


def make_plan(n):
    """Rank-banded slots + contiguous group-space layout.

    Returns (order, P, layout, T, bins) where
      order: rank -> batch id (core c slot j holds batch order[j*NC+c])
      P[j]:  uniform row count of slot j (multiple of 32; 0 = empty band)
      layout[j]: (bin, group_offset_in_bin, n_groups) for each slot P>0
      bins: list of per-bin group counts (bin = one PSUM tile, <=16 groups,
            row-range padded to a multiple of 128 except the last bin)
      T: total 128-row slices of the packed buffer R
    """
    order = np.argsort(-n, kind="stable")
    P = []
    for j in range(NB):
        nj = int(n[order[j * NCORES]])
        P.append(0 if nj == 0 else ((nj + 31) // 32) * 32)

    # first-fit-decreasing into bins of 16 groups (one PSUM bank each)
    slots = sorted(
        [(P[j] // 32, j) for j in range(NB) if P[j] > 0], reverse=True
    )
    bins = []                # per bin: [used_groups]
    binslots = []            # per bin: [(j, goff, g)]
    for g, j in slots:
        for bi in range(len(bins)):
            if bins[bi] + g <= 16:
                binslots[bi].append((j, bins[bi], g))
                bins[bi] += g
                break
        else:
            bins.append(g)
            binslots.append([(j, 0, g)])
    # pad every bin but the last to a multiple of 4 groups (128 rows)
    for bi in range(len(bins) - 1):
        bins[bi] = ((bins[bi] + 3) // 4) * 4
    layout = {}
    for bi, bs in enumerate(binslots):
        for j, goff, g in bs:
            layout[j] = (bi, goff, g)
    total_groups = sum(bins)
    T = (total_groups * 32 + 127) // 128
    return order, P, layout, T, bins


# ---------------------------------------------------------------- bass

def build_bass(P, layout, T, bins):
    nc = bacc.Bacc("TRN2", target_bir_lowering=False, debug=False)

    R = None
    if T > 0:
        R = nc.dram_tensor("spanrows", [T * 128, D], F32,
                           kind="ExternalInput").ap()
    K = nc.dram_tensor("konst", [128, KCOLS], F32, kind="ExternalInput").ap()
    out = nc.dram_tensor("out", [NB, D], F32, kind="ExternalOutput").ap()

    # bin -> (first slice t, #slices) in R
    bin_t0 = []
    t = 0
    for bi, g in enumerate(bins):
        nt = (g * 32 + 127) // 128
        bin_t0.append(t)
        t += nt
    nslots = sum(1 for p in P if p > 0)

    with tile.TileContext(nc) as tc:
        with (
            tc.tile_pool(name="persist", bufs=1) as pp,
            tc.tile_pool(name="tr", bufs=1, space="PSUM") as ppool,
        ):
            K_sb = pp.tile([128, KCOLS], F32)
            nc.scalar.dma_start(out=K_sb[:], in_=K)
            ident = K_sb[:, 0:128]
            mu_col = K_sb[0:32, 128:129]
            clsv = K_sb[0:32, CLS_OFF : CLS_OFF + NB * 32]

            if T > 0:
                # tiny warm-up transfer so the HW queues' startup latency
                # is paid before the first real slice lands in their FIFOs
                warm = pp.tile([16, 16], F32)
                nc.sync.dma_start(out=warm[:], in_=R[0:16, 0:16])

                Xs = pp.tile([128, T, D], F32)
                H = D // 2
                for tt in range(T):
                    if tt == 0 or tt == T - 1:
                        # split first/last slice by columns: halves finish
                        # ~0.7us apart, letting stage 1 start/finish earlier
                        for hh in range(2):
                            nc.sync.dma_start(
                                out=Xs[:, tt, hh * H : (hh + 1) * H],
                                in_=R[tt * 128 : (tt + 1) * 128,
                                      hh * H : (hh + 1) * H],
                            )
                    else:
                        nc.sync.dma_start(
                            out=Xs[:, tt, :],
                            in_=R[tt * 128 : (tt + 1) * 128, :],
                        )

            VEC = pp.tile([32, NB * 32], F32)
            if nslots < NB:
                nc.vector.tensor_copy(
                    VEC[:, nslots * 32 :], clsv[:, nslots * 32 :]
                )

            if T > 0:
                # psum tile per bin
                pt = [
                    ppool.tile([32, ((g * 32 + 127) // 128) * 128, ], F32,
                               tag=f"b{bi}", name=f"pt{bi}")
                    for bi, g in enumerate(bins)
                ]
                # stage 1 + transpose per slice
                s1 = [pp.tile([128, 32], F32, tag=f"r{tt}", name=f"r{tt}")
                      for tt in range(T)]
                MH = 16          # m-groups per half slice
                for tt in range(T):
                    if tt == 0 or tt == T - 1:
                        for hh in range(2):
                            nc.vector.tensor_reduce(
                                s1[tt][:, hh * MH : (hh + 1) * MH],
                                Xs[:, tt, hh * 512 : (hh + 1) * 512]
                                .rearrange("p (m c) -> p m c", c=32),
                                axis=X, op=Alu.max, apply_transpose=True,
                            )
                    else:
                        nc.vector.tensor_reduce(
                            s1[tt][:],
                            Xs[:, tt, :].rearrange("p (m c) -> p m c", c=32),
                            axis=X, op=Alu.max, apply_transpose=True,
                        )
                    # bin this slice belongs to
                    bi = max(b for b in range(len(bins)) if bin_t0[b] <= tt)
                    co = (tt - bin_t0[bi]) * 128
                    nc.tensor.transpose(
                        pt[bi][:, co : co + 128], s1[tt][:], ident
                    )

                # stage 2: per-slot contiguous group-range reduce
                for j, p in enumerate(P):
                    if p == 0:
                        continue
                    bi, goff, g = layout[j]
                    nc.vector.tensor_reduce(
                        VEC[:, j * 32 : (j + 1) * 32],
                        pt[bi][:, goff * 32 : (goff + g) * 32].rearrange(
                            "p (a i) -> p i a", a=g
                        ),
                        axis=X, op=Alu.max,
                    )

            # ---- out = cls + mu * vec ----
            OUT = pp.tile([32, NB * 32], F32)
            nc.vector.scalar_tensor_tensor(
                out=OUT[:], in0=VEC[:], scalar=mu_col,
                in1=clsv, op0=Alu.mult, op1=Alu.add,
            )
            nc.sync.dma_start(
                out=out.rearrange("b (m i) -> m b i", i=32),
                in_=OUT[:].rearrange("p (b i) -> p b i", i=32),
            )

    nc.compile()
    return nc


# ---------------------------------------------------------------- host

def make_in_maps(vector_all, ids, mu, s, n, order, P, layout, T, bins):
    va = np.asarray(vector_all, dtype=np.float32)
    muv = float(np.asarray(mu, dtype=np.float32).reshape(-1)[0])

    in_maps = []
    core_batches = []
    # slot j row range within R (groups are contiguous per bin)
    bin_row0 = []
    r = 0
    for g in bins:
        bin_row0.append(r)
        r += ((g * 32 + 127) // 128) * 128 if g != bins[-1] else g * 32
    # recompute precisely: all bins padded to x128 except last
    bin_row0 = []
    r = 0
    for bi, g in enumerate(bins):
        bin_row0.append(r)
        if bi < len(bins) - 1:
            r += ((g * 32 + 127) // 128) * 128
        else:
            r += g * 32

    for c in range(NCORES):
        batches = [int(order[j * NCORES + c]) for j in range(NB)]
        core_batches.append(batches)

        konst = np.zeros((128, KCOLS), dtype=np.float32)
        konst[:, 0:128] = np.eye(128, dtype=np.float32)
        konst[:, 128] = muv
        cls_rows = va[batches, 0, :]                    # [NB, 1024]
        konst[0:32, CLS_OFF : CLS_OFF + NB * 32] = (
            cls_rows.reshape(NB, 32, 32).transpose(1, 0, 2).reshape(32, -1)
        )
        m = {"konst": konst}

        if T > 0:
            Rbuf = np.empty((T * 128, D), dtype=np.float32)
            used = np.zeros(T * 128, dtype=bool)
            for j, p in enumerate(P):
                if p == 0:
                    continue
                bi, goff, g = layout[j]
                r0 = bin_row0[bi] + goff * 32
                b = batches[j]
                if n[b] > 0:
                    idx = s[b] + (np.arange(p) % n[b])
                else:
                    idx = np.zeros(p, dtype=np.int64)   # cls row: vec = cls
                Rbuf[r0 : r0 + p] = va[b, idx, :]
                used[r0 : r0 + p] = True
            # pad rows (never read by any slot reduce): fill with row 0
            if not used.all():
                Rbuf[~used] = va[0, 0, :]
            m["spanrows"] = Rbuf
        in_maps.append(m)
    return in_maps, core_batches


def run(vector_all, ids, mu, trace=False):
    """Returns (out [B, D] f32, BassKernelResults)."""
    s, n = compute_spans(ids)
    order, P, layout, T, bins = make_plan(n)
    nc = build_bass(P, layout, T, bins)
    in_maps, core_batches = make_in_maps(
        vector_all, ids, mu, s, n, order, P, layout, T, bins
    )
    res = run_bass_kernel_spmd(nc, in_maps, list(range(NCORES)), trace=trace)
    out = np.empty((B, D), dtype=np.float32)
    for c in range(NCORES):
        out[core_batches[c]] = res.results[c]["out"]
    return out, res


def kernel(**inputs) -> np.ndarray:
    out, _ = run(inputs["vector_all"], inputs["ids"], inputs["mu"])
    return out


# revision 12
# speedup vs baseline: 1.0819x; 1.0819x over previous
"""Trainium2 Bass kernel for nn_BiEncoder_63024350101542 (segment_reduce).

Computes, per batch row b of vector_all [B=64, L=512, D=1024]:
    mask[b,j] = (j > first_idx(ids[b]==1)) & (j < first_idx(ids[b]==2))
    span_max  = max over masked rows (fallback: CLS row 0 when mask empty)
    out[b]    = cls + mu * span_max

Only rows inside the mention span (plus the CLS row) can affect the
output, so the host shards each core's inputs as packed span windows
instead of full batches: batches are ranked by span length and dealt
round-robin into per-core slots (rank-banded), so slot j holds the same
row count on every core and one SPMD program serves all 8 cores.  Slots
are padded to a multiple of 32 rows by cycling rows of the same span
(duplicates don't change a max); empty spans are filled with the CLS
row, which makes the empty-span fallback (vec = cls) exact with no
masking at all.

Per core the padded span rows form one contiguous buffer R, streamed in
128-row slices.  Each slice gets a transpose-fused 32x32 max-reduce
(DVE) collapsing its partition groups, PE transposes land the per-group
maxima in PSUM (one bank per group of slots, split at bank boundaries),
and one tensor_reduce per slot range finishes the max.  Finally
out = cls + mu * vec, with empty slots taking cls through the same
affine for bit-exact fallback.
"""

import os
import sys

import numpy as np

for _p in ("/root/.axon_site/_ro/trn_rl_repo", "/opt/trn_rl_repo"):
    if _p not in sys.path and os.path.isdir(_p):
        sys.path.append(_p)

import concourse.bacc as bacc
import concourse.bass as bass
import concourse.mybir as mybir
import concourse.tile as tile
from concourse.bass_utils import run_bass_kernel_spmd

F32 = mybir.dt.float32
X = mybir.AxisListType.X
Alu = mybir.AluOpType

B, L, D = 64, 512, 1024
NCORES = 8
NB = B // NCORES           # batches (slots) per core
MENTION_START, MENTION_END = 1, 2

KCOLS = 392                # konst: ident 0:128, mu 128, cls rows at 136:392
CLS_OFF = 136


# ---------------------------------------------------------------- plan

def compute_spans(ids):
    """Per batch: span start s and length n (rows s..s+n-1 are masked in)."""
    ids = np.asarray(ids)
    is1 = ids == MENTION_START
    is2 = ids == MENTION_END
    first1 = np.where(is1.any(1), is1.argmax(1), L).astype(np.int64)
    first2 = np.where(is2.any(1), is2.argmax(1), L).astype(np.int64)
    s = first1 + 1
    n = np.maximum(0, first2 - s)
    return s, n


def make_plan(n):
    """Rank-banded slots packed contiguously into group space.

    Returns (order, G, go, banks, T) where
      order: rank -> batch id (core c slot j holds batch order[j*NC+c])
      G[j]:  group count of slot j (32 rows each; 0 = empty band)
      go[j]: slot j's first group in the packed buffer R
      banks: list of (first_group, n_groups) PSUM banks (<=16 groups each,
             slots never straddle a bank)
      T: number of 128-row slices of R (last may be partial)
    """
    order = np.argsort(-n, kind="stable")
    G = []
    for j in range(NB):
        nj = int(n[order[j * NCORES]])
        G.append((nj + 31) // 32)
    nonempty = [j for j in range(NB) if G[j] > 0]

    def greedy_banks(seq):
        """Greedy-fill banks of <=16 groups; interior bank boundaries must
        fall on 128-row slice boundaries (cum groups % 4 == 0) so PE
        transposes never start at partition 96 (and never straddle)."""
        go_, banks_ = {}, []
        cur_g0, cur_g, tot_ = 0, 0, 0
        for j in seq:
            if cur_g + G[j] > 16:
                if cur_g % 4 != 0:
                    return None
                banks_.append((cur_g0, cur_g))
                cur_g0, cur_g = tot_, 0
            go_[j] = tot_
            cur_g += G[j]
            tot_ += G[j]
        banks_.append((cur_g0, cur_g))
        return go_, banks_, tot_

    import itertools
    plan = greedy_banks(nonempty)
    if plan is None or len(nonempty) <= 8:
        for perm in itertools.permutations(nonempty):
            p = greedy_banks(perm)
            if p is not None:
                plan = p
                break
    if plan is None:
        # fallback: pad each bank to a multiple of 4 groups
        go, banks = {}, []
        cur_g0, cur_g, tot = 0, 0, 0
        for j in nonempty:
            if cur_g + G[j] > 16:
                pad = (-cur_g) % 4
                cur_g += pad
                tot += pad
                banks.append((cur_g0, cur_g))
                cur_g0, cur_g = tot, 0
            go[j] = tot
            cur_g += G[j]
            tot += G[j]
        banks.append((cur_g0, cur_g))
    else:
        go, banks, tot = plan
    T = (tot * 32 + 127) // 128
    return order, G, go, banks, T


# ---------------------------------------------------------------- bass

def build_bass(G, go, banks, T):
    nc = bacc.Bacc("TRN2", target_bir_lowering=False, debug=False)

    tot_g = sum(G)
    nrows = tot_g * 32
    R = None
    if T > 0:
        R = nc.dram_tensor("spanrows", [nrows, D], F32,
                           kind="ExternalInput").ap()
    K = nc.dram_tensor("konst", [128, KCOLS], F32, kind="ExternalInput").ap()
    out = nc.dram_tensor("out", [NB, D], F32, kind="ExternalOutput").ap()

    nslots = sum(1 for g in G if g > 0)

    with tile.TileContext(nc) as tc:
        with (
            tc.tile_pool(name="persist", bufs=1) as pp,
            tc.tile_pool(name="tr", bufs=1, space="PSUM") as ppool,
        ):
            K_sb = pp.tile([128, KCOLS], F32)
            nc.scalar.dma_start(out=K_sb[:], in_=K)
            ident = K_sb[:, 0:128]
            mu_col = K_sb[0:32, 128:129]
            clsv = K_sb[0:32, CLS_OFF : CLS_OFF + NB * 32]

            if T > 0:
                Xs = pp.tile([128, T, D], F32)
                for tt in range(T):
                    h = min(128, nrows - tt * 128)
                    nc.sync.dma_start(
                        out=Xs[0:h, tt, :],
                        in_=R[tt * 128 : tt * 128 + h, :],
                    )

                pt = [
                    ppool.tile([32, bg * 32], F32, tag=f"b{bi}",
                               name=f"pt{bi}")
                    for bi, (_, bg) in enumerate(banks)
                ]
                s1 = [pp.tile([128, 32], F32, tag=f"r{tt}", name=f"r{tt}")
                      for tt in range(T)]
                for tt in range(T):
                    h = min(128, nrows - tt * 128)
                    nc.vector.tensor_reduce(
                        s1[tt][0:h, :],
                        Xs[0:h, tt, :].rearrange("p (m c) -> p m c", c=32),
                        axis=X, op=Alu.max, apply_transpose=True,
                    )
                    # transpose per bank range overlapping this slice
                    g0, g1 = tt * 4, tt * 4 + (h + 31) // 32
                    for bi, (bg0, bg) in enumerate(banks):
                        lo, hi = max(g0, bg0), min(g1, bg0 + bg)
                        if lo >= hi:
                            continue
                        p0 = (lo - g0) * 32
                        p1 = (hi - g0) * 32
                        nc.tensor.transpose(
                            pt[bi][:, (lo - bg0) * 32 : (hi - bg0) * 32],
                            s1[tt][p0:p1, :],
                            ident[0 : p1 - p0, 0 : p1 - p0],
                        )

                # finish: per maximal run of equal-sized slots in one bank
                VEC = pp.tile([32, nslots * 32], F32)
                j = 0
                while j < nslots:
                    bi = next(i for i, (bg0, bg) in enumerate(banks)
                              if bg0 <= go[j] < bg0 + bg)
                    bg0, bg = banks[bi]
                    k = j + 1
                    while (k < nslots and G[k] == G[j]
                           and go[k] == go[j] + (k - j) * G[j]
                           and bg0 <= go[k] < bg0 + bg):
                        k += 1
                    ns, g = k - j, G[j]
                    co = (go[j] - bg0) * 32
                    nc.vector.tensor_reduce(
                        VEC[:, j * 32 : k * 32],
                        pt[bi][:, co : co + ns * g * 32].rearrange(
                            "p (s a i) -> p s i a", s=ns, a=g
                        ),
                        axis=X, op=Alu.max,
                    )
                    j = k

            OUT = pp.tile([32, NB * 32], F32)
            if nslots > 0:
                nc.vector.scalar_tensor_tensor(
                    out=OUT[:, : nslots * 32], in0=VEC[:], scalar=mu_col,
                    in1=clsv[:, : nslots * 32], op0=Alu.mult, op1=Alu.add,
                )
            if nslots < NB:
                # empty slots: vec = cls, same affine for bit-exactness
                nc.vector.scalar_tensor_tensor(
                    out=OUT[:, nslots * 32 :], in0=clsv[:, nslots * 32 :],
                    scalar=mu_col, in1=clsv[:, nslots * 32 :],
                    op0=Alu.mult, op1=Alu.add,
                )
            nc.sync.dma_start(
                out=out.rearrange("b (m i) -> m b i", i=32),
                in_=OUT[:].rearrange("p (b i) -> p b i", i=32),
            )

    nc.compile()
    return nc


# ---------------------------------------------------------------- host

def make_in_maps(vector_all, ids, mu, s, n, order, G, go, T):
    va = np.asarray(vector_all, dtype=np.float32)
    muv = float(np.asarray(mu, dtype=np.float32).reshape(-1)[0])
    nrows = sum(G) * 32

    in_maps = []
    core_batches = []
    for c in range(NCORES):
        batches = [int(order[j * NCORES + c]) for j in range(NB)]
        core_batches.append(batches)

        konst = np.zeros((128, KCOLS), dtype=np.float32)
        konst[:, 0:128] = np.eye(128, dtype=np.float32)
        konst[:, 128] = muv
        cls_rows = va[batches, 0, :]                    # [NB, 1024]
        konst[0:32, CLS_OFF : CLS_OFF + NB * 32] = (
            cls_rows.reshape(NB, 32, 32).transpose(1, 0, 2).reshape(32, -1)
        )
        m = {"konst": konst}

        if T > 0:
            Rbuf = np.empty((nrows, D), dtype=np.float32)
            for j in range(NB):
                if G[j] == 0:
                    continue
                r0, p = go[j] * 32, G[j] * 32
                b = batches[j]
                if n[b] > 0:
                    idx = s[b] + (np.arange(p) % n[b])
                else:
                    idx = np.zeros(p, dtype=np.int64)   # cls row: vec = cls
                Rbuf[r0 : r0 + p] = va[b, idx, :]
            m["spanrows"] = Rbuf
        in_maps.append(m)
    return in_maps, core_batches


def run(vector_all, ids, mu, trace=False):
    """Returns (out [B, D] f32, BassKernelResults)."""
    s, n = compute_spans(ids)
    order, G, go, banks, T = make_plan(n)
    nc = build_bass(G, go, banks, T)
    in_maps, core_batches = make_in_maps(
        vector_all, ids, mu, s, n, order, G, go, T
    )
    res = run_bass_kernel_spmd(nc, in_maps, list(range(NCORES)), trace=trace)
    out = np.empty((B, D), dtype=np.float32)
    for c in range(NCORES):
        out[core_batches[c]] = res.results[c]["out"]
    return out, res


def kernel(**inputs) -> np.ndarray:
    out, _ = run(inputs["vector_all"], inputs["ids"], inputs["mu"])
    return out
